# revision 14
# baseline (speedup 1.0000x reference)
"""Corr1d-x-group cost-volume kernel for Trainium2 (8 NeuronCores, SPMD).

Data-parallel over batch N=8: core i processes batch i.

Per core (inputs [16, 256, 512] f32 each, output [108, 256, 512]):
  out[g*27+ch, h, w] = 0.25 * sum_c f1[g*4+c, h, w] * f2[g*4+c, h, w+ch-23]
with zero padding outside w in [0, 512).

Structure (see git-style history in comments of earlier revisions):
  - Partition layout (c16, hl8): partition = c*8 + h//32, free = (j=h%32, w).
    f1 and a tight f2 copy are cast-loaded f32->f16 by SWDGE; the first
    j-half arrives as quarter-loads so compute starts ~12us in. ScalarE
    builds two padded f2 copies on-chip (even parity at column 24+w, odd at
    23+w) so every DVE slice start stays 4B-aligned (2x tensor_tensor
    mode); the parity builds are interleaved between early PSUM evacs so
    they never block the ScalarE FIFO. Channels using the even-parity copy
    run first so the odd build hides behind compute.
  - One VectorE tensor_mul per (ch, j-half), FD=8192 (the first two chunks
    are split into FD=4096 quarters to cut the pipeline fill).
  - Channel reduction: TensorE matmul, K=128=(c,hl), M=32=(g,hl), shared
    constant weight; 16 row-matmuls (jj4 x u4) pack one [128,2048] PSUM
    tile (4 banks) via tile_position + bank-aligned free offsets.
  - One ScalarE evac per chunk (PSUM f32 -> SBUF f16) and ONE contiguous
    512KB HWDGE store per chunk into a chunk-major DRAM tensor
    [54, 128, 2048]; the host undoes the permutation during the f16->f32
    upcast it performs anyway (rel-err ~1e-3, well under the 2e-2 gate).
"""

import os
import numpy as np

import concourse.bass as bass
import concourse.bacc as bacc
import concourse.mybir as mybir
import concourse.tile as tile
from concourse import bass_utils

N, C, H, W = 8, 16, 256, 512
G = 4
TOP_CH = 27
RADIUS = 13
PAD_SHIFT = -10  # shift s = ch - 23 for ch in [0, 27)
OUT_CH = G * TOP_CH  # 108

HL = 8          # partition sub-index: h // 32
NJ = 32         # free rows per partition: j = h % 32
PADE = 24       # f2 even tile: value f2[w] at column 24 + w
PADO = 23       # f2 odd tile:  value f2[w] at column 23 + w
F2W = 544       # padded row width (even, so row strides stay 4B-aligned)
NCHUNK = TOP_CH * 2  # (ch, j-half) chunks

_CACHED = {}


def _reduction_weights() -> np.ndarray:
    # lhsT [K=(c16,hl8)=128, M=(g4,hl8)=32]: sums the 4 channels of each
    # group and applies the 1/sumelems scale.
    w = np.zeros((128, 32), np.float16)
    for c in range(C):
        for hl in range(HL):
            w[c * HL + hl, (c // G) * HL + hl] = 0.25
    return w


def _build_program() -> bass.Bass:
    # Bacc (not raw Bass): its compile() splits multi-sem sync waits, which
    # TRN2 hardware limits to one per instruction.
    nc = bacc.Bacc(
        "TRN2",
        target_bir_lowering=False,
        debug=False,
        enable_asserts=False,
        num_devices=N,
    )
    f16 = mybir.dt.float16
    f32 = mybir.dt.float32

    l_in = nc.dram_tensor("l_in", [C, H, W], f32, kind="ExternalInput")
    r_in = nc.dram_tensor("r_in", [C, H, W], f32, kind="ExternalInput")
    w_red = nc.dram_tensor("w_red", [128, 32], f16, kind="ExternalInput")
    # Chunk-major output: [chunk=(ch,jh), partition=(jj,g,hl), (u,w)].
    out = nc.dram_tensor("out", [NCHUNK, 128, 4 * W], f16, kind="ExternalOutput")

    l_src = l_in.ap().rearrange("c (hl j) w -> (c hl) j w", hl=HL)
    r_src = r_in.ap().rearrange("c (hl j) w -> (c hl) j w", hl=HL)

    # Process even-parity shifts (f2e) first so the on-chip f2o build
    # (ScalarE) hides behind compute. col0 = PADE + s must be even for f2e.
    ch_even_par = [ch for ch in range(TOP_CH) if (PADE + ch - 23) % 2 == 0]
    ch_odd_par = [ch for ch in range(TOP_CH) if (PADE + ch - 23) % 2 == 1]
    ch_order = ch_even_par + ch_odd_par

    with tile.TileContext(nc) as tc:
        with (
            tc.tile_pool(name="wpool", bufs=1) as wpool,
            tc.tile_pool(name="inpool", bufs=1) as inpool,
            tc.tile_pool(name="f2tpool", bufs=1) as f2tpool,
            tc.tile_pool(name="prodpool", bufs=3) as prodpool,
            tc.tile_pool(name="obpool", bufs=3) as obpool,
            tc.tile_pool(name="psumpool", bufs=2, space="PSUM") as psumpool,
        ):
            wt = wpool.tile([128, 32], f16)
            nc.sync.dma_start(wt[:], w_red[:])

            f1 = inpool.tile([128, NJ * W], f16)
            f2e = inpool.tile([128, NJ * F2W], f16)
            f2o = inpool.tile([128, NJ * F2W], f16)
            f1v = f1.rearrange("p (j w) -> p j w", w=W)
            f2ev = f2e.rearrange("p (j x) -> p j x", x=F2W)
            f2ov = f2o.rearrange("p (j x) -> p j x", x=F2W)

            # Static zero padding left/right of each 544-column row.
            nc.vector.memset(f2ev[:, :, 0:PADE], 0.0)
            nc.vector.memset(f2ev[:, :, PADE + W : F2W], 0.0)
            nc.vector.memset(f2ov[:, :, 0:PADO], 0.0)
            nc.vector.memset(f2ov[:, :, PADO + W : F2W], 0.0)

            # Tight f2 staging tile (dead once the parity copies exist);
            # reused for half 1 -- the WAR dependency delays the half-1
            # load until the half-0 parity builds are done (~25us), still
            # ~90us before half-1 compute needs it.
            f2t = f2tpool.tile([128, 16 * W], f16, name="f2t")
            f2tv = f2t.rearrange("p (j w) -> p j w", w=W)

            # Cast-loads for half 0, as interleaved quarters so the first
            # muls can start early. (Half 1 is emitted inside the loop,
            # after the half-0 parity builds, so the WAR on f2t resolves in
            # program order.)
            for q in range(2):
                a, b = 8 * q, 8 * (q + 1)
                nc.gpsimd.dma_start(f2tv[:, a:b, :], r_src[:, a:b, :])
                nc.gpsimd.dma_start(f1v[:, a:b, :], l_src[:, a:b, :])

            # Parity builds for half 0 (ScalarE; even copy runs in fast
            # aligned mode, odd copy is 1x but off the critical path).
            for q in range(2):
                a, b = 8 * q, 8 * (q + 1)
                nc.scalar.copy(
                    f2ev[:, a:b, PADE : PADE + W], f2tv[:, a:b, :]
                )
            # Deferred work interleaved between chunk evacs:
            # chunk index -> emit callback. The f2o half-0 builds and the
            # half-1 reload stay in this program order so the single-tile
            # WAR on f2t resolves correctly.
            def _load_h1():
                nc.gpsimd.dma_start(f2tv[:, :, :], r_src[:, 16:32, :])
                nc.gpsimd.dma_start(f1v[:, 16:32, :], l_src[:, 16:32, :])

            def _build(dv, r0, r1, sr):
                pad = PADO if dv is f2ov else PADE
                nc.scalar.copy(
                    dv[:, r0:r1, pad : pad + W],
                    f2tv[:, sr : sr + (r1 - r0), :],
                )

            deferred = {
                0: lambda: _build(f2ov, 0, 8, 0),     # f2o half0 q0
                1: lambda: _build(f2ov, 8, 16, 8),    # f2o half0 q1
                2: lambda: _load_h1(),
                9: lambda: _build(f2ev, 16, 32, 0),   # f2e half1
                10: lambda: _build(f2ov, 16, 24, 0),  # f2o half1 q0
                11: lambda: _build(f2ov, 24, 32, 8),  # f2o half1 q1
            }

            chunk_idx = 0
            for jh in range(2):
                j0 = 16 * jh
                for ch in ch_order:
                    s = ch - (RADIUS - PAD_SHIFT)  # in [-23, 3]
                    if (PADE + s) % 2 == 0:
                        src3, col0 = f2ev, PADE + s
                    else:
                        src3, col0 = f2ov, PADO + s
                    p = prodpool.tile([128, 16 * W], f16, tag="prod", name="p")
                    p3 = p.rearrange("p (j w) -> p j w", w=W)
                    if chunk_idx < 2:
                        # Quarter-muls: lets the first matmuls start after
                        # the first quarter-load lands.
                        for qq in range(2):
                            a, b = 8 * qq, 8 * (qq + 1)
                            nc.vector.tensor_mul(
                                p3[:, a:b, :],
                                f1v[:, j0 + a : j0 + b, :],
                                src3[:, j0 + a : j0 + b, col0 : col0 + W],
                            )
                    else:
                        nc.vector.tensor_mul(
                            p3[:, :, :],
                            f1v[:, j0 : j0 + 16, :],
                            src3[:, j0 : j0 + 16, col0 : col0 + W],
                        )
                    # 16 row-windows -> one [128, 2048] PSUM tile (4 banks):
                    # partition block jj via tile_position, bank u via the
                    # free offset. jj-major so equal positions repeat.
                    psumt = psumpool.tile([128, 4 * W], f32, tag="ps", name="ps")
                    for jj in range(4):
                        for u in range(4):
                            jl = jj * 4 + u
                            nc.tensor.matmul(
                                psumt[
                                    32 * jj : 32 * (jj + 1),
                                    W * u : W * (u + 1),
                                ],
                                wt[:],
                                p[:, W * jl : W * (jl + 1)],
                                start=True,
                                stop=True,
                                tile_position=(0, 32 * jj),
                            )
                    ob = obpool.tile([128, 4 * W], f16, tag="ob", name="ob")
                    nc.scalar.copy(ob[:], psumt[:])
                    nc.sync.dma_start(
                        out.ap()[ch * 2 + jh : ch * 2 + jh + 1], ob[:]
                    )
                    if chunk_idx in deferred:
                        deferred[chunk_idx]()
                    chunk_idx += 1
    nc.compile()
    return nc


def kernel(l_in: np.ndarray, r_in: np.ndarray) -> np.ndarray:
    assert l_in.shape == (N, C, H, W) and r_in.shape == (N, C, H, W)
    l_in = np.ascontiguousarray(l_in, dtype=np.float32)
    r_in = np.ascontiguousarray(r_in, dtype=np.float32)

    if "nc" not in _CACHED:
        _CACHED["nc"] = _build_program()
    nc = _CACHED["nc"]

    w_np = _reduction_weights()
    in_maps = [
        {
            "l_in": np.ascontiguousarray(l_in[i]),
            "r_in": np.ascontiguousarray(r_in[i]),
            "w_red": w_np,
        }
        for i in range(N)
    ]
    trace = bool(int(os.environ.get("CORR_KERNEL_TRACE", "0")))
    kwargs = {}
    tdir = os.environ.get("CORR_KERNEL_TRACE_DIR")
    if trace and tdir:
        os.makedirs(tdir, exist_ok=True)
        kwargs["tmpdir"] = tdir
    res = bass_utils.run_bass_kernel_spmd(
        nc, in_maps, core_ids=list(range(N)), trace=trace, **kwargs
    )
    _CACHED["last_result"] = res

    # Undo the device's chunk-major layout while upcasting to f32:
    # out_d[ch, jh, jj, g, hl, u, w] -> out[g*27+ch, hl*32+jh*16+jj*4+u, w].
    outs = []
    for i in range(N):
        x = res.results[i]["out"].reshape(TOP_CH, 2, 4, G, HL, 4, W)
        x = x.transpose(3, 0, 4, 1, 2, 5, 6).reshape(OUT_CH, H, W)
        outs.append(x)
    return np.stack(outs, axis=0).astype(np.float32)
